# revision 30
# baseline (speedup 1.0000x reference)
"""AttentionMemory kernel for Trainium2 (8 NeuronCores, Bass/Tile).

Reference computation (per batch b):
    affinity[n, m] = (2 * mk[:,n]@qk[:,m] - ||mk[:,n]||^2 - ||qk[:,m]||^2) / 8
    out[n, m]      = softmax over n (memory axis)

Logits come from a single float32r (tf32-speed) augmented matmul:
    lhsT (stationary) = [0.25 * qk ; -0.125]      -> [65, Mc]
    rhs  (moving)     = [mk        ; a_n - abar]  -> [65, N]
    psum[m, n]        = 0.25*qk_m.mk_n - 0.125*(a_n - abar)
with a_n = sum_c mk[c,n]^2 precomputed on the host.  The ACT exp pass adds a
per-partition bias -0.125*(||qk_m||^2 + abar), making the exp argument exactly
-||mk_n - qk_m||^2 / 8 <= 0: no overflow, and per-column constants cancel in
the softmax.  float32r rounds inputs to ~tf32; simulated end-to-end absmax
error is ~2.3e-3 of scale (gate 2e-2).

exp values and the normalized output are staged in bf16 (DVE runs 4x in
16-bit, DMA stores are half-size; bf16's f32-range exponent avoids the
denormal flushing fp16 would hit on tiny softmax outputs); the host
upconverts to f32 during the gather/transpose.  Row sums ride the ACT accumulator (free), reciprocal +
scale on DVE.

Sharding: core c handles batch c//2, query-column half c%2 (communication
free: softmax is over the full n axis which each core holds).  Each core
writes out_c[m, n] bf16; the host transposes to the reference [n, m] layout.

Per-core budget (TimelineSim cost model): ACT exp 0.833ns/col * 64512 cols
~= 60us (bottleneck), fp16 stores 45us, PE f32r 27-54us, DVE ~39us.
"""

import numpy as np

B, CK, H, W = 4, 64, 48, 84
N = H * W            # 4032 memory pixels (softmax axis)
HALF = N // 2        # 2016 query pixels per core
M_STRIP = 126        # output-partition strip size (16 * 126 = 2016)
N_STRIPS = HALF // M_STRIP
K_AUG = CK + 1       # 65: contraction dim incl. the (a_n - abar) row

N_CHUNK = 504        # matmul moving free dim; 4 chunks per 4-bank PSUM piece
PIECE = 4 * N_CHUNK  # 2016 cols per ACT exp call
N_PIECES = N // PIECE  # 2

_CACHE = {}

# scheduler-lottery knobs (neutral semantics, reshuffle the tile scheduler);
# defaults are the shipped configuration
import os as _os
_SPINS = int(_os.environ.get("K_SPINS", "14"))
_EXPB = int(_os.environ.get("K_EXPB", "3"))
_OUTB = int(_os.environ.get("K_OUTB", "3"))
_SYNCM = int(_os.environ.get("K_SYNCM", "2"))
_S0 = _os.environ.get("K_S0", "224")


def _build_nc():
    import concourse.bacc as bacc
    import concourse.mybir as mybir
    import concourse.tile as tile

    f32 = mybir.dt.float32
    f32r = mybir.dt.float32r
    bf16 = mybir.dt.bfloat16
    f16 = mybir.dt.float16
    Exp = mybir.ActivationFunctionType.Exp

    nc = bacc.Bacc("TRN2", target_bir_lowering=False, debug=False)

    q_d = nc.dram_tensor("q2", [K_AUG, HALF], f32r, kind="ExternalInput")
    m_d = nc.dram_tensor("m2", [K_AUG, N], f32r, kind="ExternalInput")
    b_d = nc.dram_tensor("bias", [M_STRIP, N_STRIPS], f32, kind="ExternalInput")
    out_d = nc.dram_tensor("out_c", [HALF, N], bf16, kind="ExternalOutput")

    with tile.TileContext(nc) as tc:
        with (
            tc.tile_pool(name="singles", bufs=1) as singles,
            tc.tile_pool(name="psum", bufs=2, space="PSUM") as psum_pool,
            tc.tile_pool(name="exp", bufs=_EXPB) as exp_pool,
            tc.tile_pool(name="outs", bufs=_OUTB) as out_pool,
            tc.tile_pool(name="stats", bufs=8) as stats_pool,
        ):
            # --- prewarm: ACT exp table load + PE pstate ramp during the
            # input DMAs -----------------------------------------------------
            wtab = singles.tile([1, 2], f32)
            nc.vector.memset(wtab, 0.0)
            nc.scalar.activation(wtab[:, 1:2], wtab[:, 0:1], Exp)
            wsrc = singles.tile([K_AUG, 256], bf16)
            nc.vector.memset(wsrc, 0.0)
            wps = psum_pool.tile([M_STRIP, 2048], f32, tag="ps")
            for _ in range(_SPINS):
                nc.tensor.matmul(
                    wps[:, :256], wsrc[:, :M_STRIP], wsrc, start=True, stop=True
                )

            # --- inputs, staged by first use.  SP ring: bias (tiny, gates the
            # first exp) + strip-0 q + first two m chunks + rest of q; Pool
            # ring (SWDGE, otherwise idle) carries the remaining m chunks so
            # the two sequencers dispatch concurrently and the ACT sequencer
            # stays free for exp dispatches --------------------------------
            q_s = singles.tile([K_AUG, HALF], f32r)
            m_s = singles.tile([K_AUG, N], f32r)
            b_s = singles.tile([M_STRIP, N_STRIPS], f32)
            nc.sync.dma_start(out=b_s, in_=b_d[:, :])
            nc.sync.dma_start(out=q_s[:, :M_STRIP], in_=q_d[:, :M_STRIP])
            for c in range(_SYNCM):
                sl = slice(c * N_CHUNK, (c + 1) * N_CHUNK)
                nc.sync.dma_start(out=m_s[:, sl], in_=m_d[:, sl])
            for c in range(_SYNCM, 8):
                sl = slice(c * N_CHUNK, (c + 1) * N_CHUNK)
                nc.gpsimd.dma_start(out=m_s[:, sl], in_=m_d[:, sl])
            nc.scalar.dma_start(out=q_s[:, M_STRIP:], in_=q_d[:, M_STRIP:])

            for s in range(N_STRIPS):
                m0 = s * M_STRIP
                q_l = q_s[:, m0 : m0 + M_STRIP]

                # strip 0 exps in small leading pieces so the ACT stream
                # starts as soon as the first m chunks land; steady state
                # uses 2016-col pieces (fewer per-call overheads)
                if s == 0 and _S0 == "224":
                    pieces = [range(0, 2), range(2, 4), range(4, 8)]
                elif s == 0 and _S0 == "2222":
                    pieces = [range(0, 2), range(2, 4), range(4, 6), range(6, 8)]
                else:
                    pieces = [range(0, 4), range(4, 8)]

                exp_t = exp_pool.tile([M_STRIP, N], bf16, tag="exp")
                acc = stats_pool.tile([M_STRIP, len(pieces)], f32, tag="acc")

                for pi, piece in enumerate(pieces):
                    k = len(piece)
                    # 1 PSUM bank (512 f32) per 504-col chunk; chunks start on
                    # bank boundaries so PE writes never straddle one
                    ps = psum_pool.tile([M_STRIP, 512 * k], f32, tag="ps")
                    for cc, c in enumerate(piece):
                        nc.tensor.matmul(
                            ps[:, cc * 512 : cc * 512 + N_CHUNK],
                            q_l,
                            m_s[:, c * N_CHUNK : (c + 1) * N_CHUNK],
                            start=True,
                            stop=True,
                        )
                    # exp(logits + bias_m) PSUM->SBUF fp16 with fused
                    # per-partition row sum; 3D views skip the 8 pad cols/bank
                    e0 = piece[0] * N_CHUNK
                    nc.scalar.activation(
                        exp_t[:, e0 : e0 + k * N_CHUNK].rearrange(
                            "p (b c) -> p b c", b=k
                        ),
                        ps.rearrange("p (b c) -> p b c", b=k)[:, :, :N_CHUNK],
                        Exp,
                        bias=b_s[:, s : s + 1],
                        accum_out=acc[:, pi : pi + 1],
                    )

                ssum = stats_pool.tile([M_STRIP, 1], f32, tag="ssum")
                nc.vector.reduce_sum(ssum, acc, axis=mybir.AxisListType.X)
                rcp = stats_pool.tile([M_STRIP, 1], f32, tag="rcp")
                nc.vector.reciprocal(rcp, ssum)

                out_t = out_pool.tile([M_STRIP, N], bf16, tag="out")
                if s == 0:
                    # quarters so the first bytes hit the DMA ring early
                    tsm_bounds = [0, 1008, 2016, 3024, N]
                    store_bounds = tsm_bounds
                elif s == N_STRIPS - 1:
                    # quartered scale + stores shorten the drain tail
                    tsm_bounds = [0, 1008, 2016, 3024, N]
                    store_bounds = tsm_bounds
                else:
                    tsm_bounds = [0, N]
                    store_bounds = [0, 2016, N]
                tsm_spans = dict(zip(tsm_bounds, tsm_bounds[1:]))
                for p0, p1 in zip(store_bounds, store_bounds[1:]):
                    if p0 in tsm_spans:
                        t1 = tsm_spans[p0]
                        nc.vector.tensor_scalar_mul(
                            out_t[:, p0:t1], exp_t[:, p0:t1], rcp
                        )
                    nc.sync.dma_start(
                        out=out_d[m0 : m0 + M_STRIP, p0:p1],
                        in_=out_t[:, p0:p1],
                    )

    nc.compile()
    return nc


def _get_nc():
    if "nc" not in _CACHE:
        _CACHE["nc"] = _build_nc()
    return _CACHE["nc"]


def _round_tf32(x: np.ndarray) -> np.ndarray:
    """Round f32 to 11-bit mantissa (tf32/f32r) with round-to-nearest."""
    xi = np.ascontiguousarray(x, dtype=np.float32).view(np.uint32)
    return ((xi + np.uint32(0x1000)) & np.uint32(0xFFFFE000)).view(np.float32)


def kernel(mk: np.ndarray, qk: np.ndarray) -> np.ndarray:
    from concourse import bass_utils

    mk = np.asarray(mk, dtype=np.float32).reshape(B, CK, N)
    qk = np.asarray(qk, dtype=np.float32).reshape(B, CK, N)
    a = np.einsum("bcn,bcn->bn", mk, mk)        # ||mk_n||^2, [B, N]
    cq = np.einsum("bcm,bcm->bm", qk, qk)       # ||qk_m||^2, [B, M]
    abar = a.mean(axis=1)                       # [B]

    in_maps = []
    for core in range(8):
        b, h = divmod(core, 2)
        m2 = np.empty((K_AUG, N), np.float32)
        m2[:CK] = mk[b]
        m2[CK] = a[b] - abar[b]
        m2 = _round_tf32(m2)

        q2 = np.empty((K_AUG, HALF), np.float32)
        q2[:CK] = 0.25 * qk[b, :, h * HALF : (h + 1) * HALF]
        q2[CK] = -0.125
        q2 = _round_tf32(q2)

        bias = (
            (-0.125 * (cq[b, h * HALF : (h + 1) * HALF] + abar[b]))
            .astype(np.float32)
            .reshape(N_STRIPS, M_STRIP)
            .T.copy()
        )
        in_maps.append({"q2": q2, "m2": m2, "bias": bias})

    res = bass_utils.run_bass_kernel_spmd(
        _get_nc(), in_maps, core_ids=list(range(8))
    )
    _CACHE["last_results"] = res

    out = np.empty((B, N, N), np.float32)
    for core in range(8):
        b, h = divmod(core, 2)
        out[b, :, h * HALF : (h + 1) * HALF] = (
            res.results[core]["out_c"].T.astype(np.float32)
        )
    return out


# revision 34
# speedup vs baseline: 1.0341x; 1.0341x over previous
"""AttentionMemory kernel for Trainium2 (8 NeuronCores, Bass/Tile).

Reference computation (per batch b):
    affinity[n, m] = (2 * mk[:,n]@qk[:,m] - ||mk[:,n]||^2 - ||qk[:,m]||^2) / 8
    out[n, m]      = softmax over n (memory axis)

Logits come from a single float32r (tf32-speed) augmented matmul:
    lhsT (stationary) = [0.25 * qk ; -0.125]      -> [65, Mc]
    rhs  (moving)     = [mk        ; a_n - abar]  -> [65, N]
    psum[m, n]        = 0.25*qk_m.mk_n - 0.125*(a_n - abar)
with a_n = sum_c mk[c,n]^2 precomputed on the host.  The ACT exp pass adds a
per-partition bias -0.125*(||qk_m||^2 + abar), making the exp argument exactly
-||mk_n - qk_m||^2 / 8 <= 0: no overflow, and per-column constants cancel in
the softmax.  float32r rounds inputs to ~tf32; simulated end-to-end absmax
error is ~2.3e-3 of scale (gate 2e-2).

exp values and the normalized output are staged in bf16 (DVE runs 4x in
16-bit, DMA stores are half-size; bf16's f32-range exponent avoids the
denormal flushing fp16 would hit on tiny softmax outputs); the host
upconverts to f32 during the gather/transpose.  Row sums ride the ACT accumulator (free), reciprocal +
scale on DVE.

Sharding: core c handles batch c//2, query-column half c%2 (communication
free: softmax is over the full n axis which each core holds).  Each core
writes out_c[m, n] bf16; the host transposes to the reference [n, m] layout.

Per-core budget (TimelineSim cost model): ACT exp 0.833ns/col * 64512 cols
~= 60us (bottleneck), fp16 stores 45us, PE f32r 27-54us, DVE ~39us.
"""

import numpy as np

B, CK, H, W = 4, 64, 48, 84
N = H * W            # 4032 memory pixels (softmax axis)
HALF = N // 2        # 2016 query pixels per core
M_STRIP = 126        # output-partition strip size (16 * 126 = 2016)
N_STRIPS = HALF // M_STRIP
K_AUG = CK + 1       # 65: contraction dim incl. the (a_n - abar) row

N_CHUNK = 504        # matmul moving free dim; 4 chunks per 4-bank PSUM piece
PIECE = 4 * N_CHUNK  # 2016 cols per ACT exp call
N_PIECES = N // PIECE  # 2

_CACHE = {}

# scheduler-lottery knobs (neutral semantics, reshuffle the tile scheduler);
# defaults are the shipped configuration
import os as _os
_SPINS = int(_os.environ.get("K_SPINS", "14"))
_EXPB = int(_os.environ.get("K_EXPB", "3"))
_OUTB = int(_os.environ.get("K_OUTB", "3"))
_SYNCM = int(_os.environ.get("K_SYNCM", "2"))
_S0 = _os.environ.get("K_S0", "224")


def _build_nc():
    import concourse.bacc as bacc
    import concourse.mybir as mybir
    import concourse.tile as tile

    f32 = mybir.dt.float32
    f32r = mybir.dt.float32r
    bf16 = mybir.dt.bfloat16
    f16 = mybir.dt.float16
    Exp = mybir.ActivationFunctionType.Exp

    nc = bacc.Bacc("TRN2", target_bir_lowering=False, debug=False)

    q_d = nc.dram_tensor("q2", [K_AUG, HALF], f32r, kind="ExternalInput")
    m_d = nc.dram_tensor("m2", [K_AUG, N], f32r, kind="ExternalInput")
    b_d = nc.dram_tensor("bias", [M_STRIP, N_STRIPS], f32, kind="ExternalInput")
    out_d = nc.dram_tensor("out_c", [HALF, N], bf16, kind="ExternalOutput")

    with tile.TileContext(nc) as tc:
        with (
            tc.tile_pool(name="singles", bufs=1) as singles,
            tc.tile_pool(name="psum", bufs=2, space="PSUM") as psum_pool,
            tc.tile_pool(name="exp", bufs=_EXPB) as exp_pool,
            tc.tile_pool(name="outs", bufs=_OUTB) as out_pool,
            tc.tile_pool(name="stats", bufs=8) as stats_pool,
        ):
            # --- prewarm: ACT exp table load + PE pstate ramp during the
            # input DMAs -----------------------------------------------------
            wtab = singles.tile([1, 2], f32)
            nc.vector.memset(wtab, 0.0)
            nc.scalar.activation(wtab[:, 1:2], wtab[:, 0:1], Exp)
            wsrc = singles.tile([K_AUG, 256], bf16)
            nc.vector.memset(wsrc, 0.0)
            wps = psum_pool.tile([M_STRIP, 2048], f32, tag="ps")
            for _ in range(_SPINS):
                nc.tensor.matmul(
                    wps[:, :256], wsrc[:, :M_STRIP], wsrc, start=True, stop=True
                )

            # --- inputs, staged by first use.  SP ring: bias (tiny, gates the
            # first exp) + strip-0 q + first two m chunks + rest of q; Pool
            # ring (SWDGE, otherwise idle) carries the remaining m chunks so
            # the two sequencers dispatch concurrently and the ACT sequencer
            # stays free for exp dispatches --------------------------------
            q_s = singles.tile([K_AUG, HALF], f32r)
            m_s = singles.tile([K_AUG, N], f32r)
            b_s = singles.tile([M_STRIP, N_STRIPS], f32)
            nc.sync.dma_start(out=b_s, in_=b_d[:, :])
            nc.sync.dma_start(out=q_s[:, :M_STRIP], in_=q_d[:, :M_STRIP])
            for c in range(_SYNCM):
                sl = slice(c * N_CHUNK, (c + 1) * N_CHUNK)
                nc.sync.dma_start(out=m_s[:, sl], in_=m_d[:, sl])
            for c in range(_SYNCM, 8):
                sl = slice(c * N_CHUNK, (c + 1) * N_CHUNK)
                nc.gpsimd.dma_start(out=m_s[:, sl], in_=m_d[:, sl])
            _qr_ring = {"sp": nc.sync, "act": nc.scalar, "dve": nc.vector}[
                _os.environ.get("K_QR", "sp")
            ]
            _qr_ring.dma_start(out=q_s[:, M_STRIP:], in_=q_d[:, M_STRIP:])

            for s in range(N_STRIPS):
                m0 = s * M_STRIP
                q_l = q_s[:, m0 : m0 + M_STRIP]

                # strip 0 exps in small leading pieces so the ACT stream
                # starts as soon as the first m chunks land; steady state
                # uses 2016-col pieces (fewer per-call overheads)
                if s == 0 and _S0 == "224":
                    pieces = [range(0, 2), range(2, 4), range(4, 8)]
                elif s == 0 and _S0 == "2222":
                    pieces = [range(0, 2), range(2, 4), range(4, 6), range(6, 8)]
                else:
                    pieces = [range(0, 4), range(4, 8)]

                # middle strips skip the ACT accumulator read (187ns/call)
                # on the first piece: DVE re-reduces those exp values from
                # SBUF instead (DVE has slack; ACT is the bottleneck).  Edge
                # strips keep the fused accum on every piece (short tail).
                dve_red = 0 < s < N_STRIPS - 1
                exp_t = exp_pool.tile([M_STRIP, N], bf16, tag="exp")
                acc = stats_pool.tile([M_STRIP, len(pieces)], f32, tag="acc")

                for pi, piece in enumerate(pieces):
                    k = len(piece)
                    # 1 PSUM bank (512 f32) per 504-col chunk; chunks start on
                    # bank boundaries so PE writes never straddle one
                    ps = psum_pool.tile([M_STRIP, 512 * k], f32, tag="ps")
                    for cc, c in enumerate(piece):
                        nc.tensor.matmul(
                            ps[:, cc * 512 : cc * 512 + N_CHUNK],
                            q_l,
                            m_s[:, c * N_CHUNK : (c + 1) * N_CHUNK],
                            start=True,
                            stop=True,
                        )
                    # exp(logits + bias_m) PSUM->SBUF bf16 with fused
                    # per-partition row sum; 3D views skip the 8 pad cols/bank
                    e0 = piece[0] * N_CHUNK
                    nc.scalar.activation(
                        exp_t[:, e0 : e0 + k * N_CHUNK].rearrange(
                            "p (b c) -> p b c", b=k
                        ),
                        ps.rearrange("p (b c) -> p b c", b=k)[:, :, :N_CHUNK],
                        Exp,
                        bias=b_s[:, s : s + 1],
                        accum_out=None if (dve_red and pi == 0) else acc[:, pi : pi + 1],
                    )

                ssum = stats_pool.tile([M_STRIP, 1], f32, tag="ssum")
                if dve_red:
                    ared = stats_pool.tile([M_STRIP, 1], f32, tag="ared")
                    nc.vector.reduce_sum(
                        ared, exp_t[:, :PIECE], axis=mybir.AxisListType.X
                    )
                    nc.vector.tensor_add(ssum, ared, acc[:, 1:2])
                else:
                    nc.vector.reduce_sum(ssum, acc, axis=mybir.AxisListType.X)
                rcp = stats_pool.tile([M_STRIP, 1], f32, tag="rcp")
                nc.vector.reciprocal(rcp, ssum)

                out_t = out_pool.tile([M_STRIP, N], bf16, tag="out")
                if s == 0:
                    # quarters so the first bytes hit the DMA ring early
                    tsm_bounds = [0, 1008, 2016, 3024, N]
                    store_bounds = tsm_bounds
                elif s == N_STRIPS - 1:
                    # quartered scale + stores shorten the drain tail
                    tsm_bounds = [0, 1008, 2016, 3024, N]
                    store_bounds = tsm_bounds
                else:
                    tsm_bounds = [0, N]
                    store_bounds = [0, 2016, N]
                tsm_spans = dict(zip(tsm_bounds, tsm_bounds[1:]))
                for p0, p1 in zip(store_bounds, store_bounds[1:]):
                    if p0 in tsm_spans:
                        t1 = tsm_spans[p0]
                        nc.vector.tensor_scalar_mul(
                            out_t[:, p0:t1], exp_t[:, p0:t1], rcp
                        )
                    nc.sync.dma_start(
                        out=out_d[m0 : m0 + M_STRIP, p0:p1],
                        in_=out_t[:, p0:p1],
                    )

    nc.compile()
    return nc


def _get_nc():
    if "nc" not in _CACHE:
        _CACHE["nc"] = _build_nc()
    return _CACHE["nc"]


def _round_tf32(x: np.ndarray) -> np.ndarray:
    """Round f32 to 11-bit mantissa (tf32/f32r) with round-to-nearest."""
    xi = np.ascontiguousarray(x, dtype=np.float32).view(np.uint32)
    return ((xi + np.uint32(0x1000)) & np.uint32(0xFFFFE000)).view(np.float32)


def kernel(mk: np.ndarray, qk: np.ndarray) -> np.ndarray:
    from concourse import bass_utils

    mk = np.asarray(mk, dtype=np.float32).reshape(B, CK, N)
    qk = np.asarray(qk, dtype=np.float32).reshape(B, CK, N)
    a = np.einsum("bcn,bcn->bn", mk, mk)        # ||mk_n||^2, [B, N]
    cq = np.einsum("bcm,bcm->bm", qk, qk)       # ||qk_m||^2, [B, M]
    abar = a.mean(axis=1)                       # [B]

    in_maps = []
    for core in range(8):
        b, h = divmod(core, 2)
        m2 = np.empty((K_AUG, N), np.float32)
        m2[:CK] = mk[b]
        m2[CK] = a[b] - abar[b]
        m2 = _round_tf32(m2)

        q2 = np.empty((K_AUG, HALF), np.float32)
        q2[:CK] = 0.25 * qk[b, :, h * HALF : (h + 1) * HALF]
        q2[CK] = -0.125
        q2 = _round_tf32(q2)

        bias = (
            (-0.125 * (cq[b, h * HALF : (h + 1) * HALF] + abar[b]))
            .astype(np.float32)
            .reshape(N_STRIPS, M_STRIP)
            .T.copy()
        )
        in_maps.append({"q2": q2, "m2": m2, "bias": bias})

    res = bass_utils.run_bass_kernel_spmd(
        _get_nc(), in_maps, core_ids=list(range(8))
    )
    _CACHE["last_results"] = res

    out = np.empty((B, N, N), np.float32)
    for core in range(8):
        b, h = divmod(core, 2)
        out[b, :, h * HALF : (h + 1) * HALF] = (
            res.results[core]["out_c"].T.astype(np.float32)
        )
    return out


# revision 38
# speedup vs baseline: 1.0465x; 1.0120x over previous
"""AttentionMemory kernel for Trainium2 (8 NeuronCores, Bass/Tile).

Reference computation (per batch b):
    affinity[n, m] = (2 * mk[:,n]@qk[:,m] - ||mk[:,n]||^2 - ||qk[:,m]||^2) / 8
    out[n, m]      = softmax over n (memory axis)

Logits come from a single float32r (tf32-speed) augmented matmul:
    lhsT (stationary) = [0.25 * qk ; -0.125]      -> [65, Mc]
    rhs  (moving)     = [mk        ; a_n - abar]  -> [65, N]
    psum[m, n]        = 0.25*qk_m.mk_n - 0.125*(a_n - abar)
with a_n = sum_c mk[c,n]^2 precomputed on the host.  The ACT exp pass adds a
per-partition bias -0.125*(||qk_m||^2 + abar), making the exp argument exactly
-||mk_n - qk_m||^2 / 8 <= 0: no overflow, and per-column constants cancel in
the softmax.  float32r rounds inputs to ~tf32 (pre-rounded on the host so
the declared dtype is honest); measured end-to-end absmax error is 3.1e-3
of scale (gate 2e-2).

exp values and the normalized output are staged in bf16 (DVE runs 4x in
16-bit, DMA stores are half-size; bf16's f32-range exponent avoids the
denormal flushing fp16 would hit on tiny softmax outputs); the host
upconverts to f32 during the gather/transpose.  Row sums ride the ACT accumulator (free), reciprocal +
scale on DVE.

Sharding: core c handles batch c//2, query-column half c%2 (communication
free: softmax is over the full n axis which each core holds).  Each core
writes out_c[m, n] bf16; the host transposes to the reference [n, m] layout.

Middle strips skip the ACT accumulator read (187ns/call) on their first
2016-col piece; DVE re-reduces those exp values from SBUF instead (DVE has
~25us of slack, ACT is the bottleneck engine).

Per-core budget (TimelineSim cost model): ACT exp stream ~64.7us busy and
gap-free (bottleneck), bf16 stores + f32 loads ~49.7us DMA, PE f32r ~30us,
DVE ~49us.  Total 75.7us = 5.2us startup + ACT stream + 6.5us drain tail
(vs 104.5us for the f32/bf16-pair baseline).
"""

import numpy as np

B, CK, H, W = 4, 64, 48, 84
N = H * W            # 4032 memory pixels (softmax axis)
HALF = N // 2        # 2016 query pixels per core
M_STRIP = 126        # output-partition strip size (16 * 126 = 2016)
N_STRIPS = HALF // M_STRIP
K_AUG = CK + 1       # 65: contraction dim incl. the (a_n - abar) row

N_CHUNK = 504        # matmul moving free dim; 4 chunks per 4-bank PSUM piece
PIECE = 4 * N_CHUNK  # 2016 cols per ACT exp call
N_PIECES = N // PIECE  # 2

_CACHE = {}

# scheduler-lottery knobs (neutral semantics, reshuffle the tile scheduler);
# defaults are the shipped configuration
import os as _os
_SPINS = int(_os.environ.get("K_SPINS", "14"))
_EXPB = int(_os.environ.get("K_EXPB", "3"))
_OUTB = int(_os.environ.get("K_OUTB", "3"))
_SYNCM = int(_os.environ.get("K_SYNCM", "2"))
_S0 = _os.environ.get("K_S0", "2222")


def _build_nc():
    import concourse.bacc as bacc
    import concourse.mybir as mybir
    import concourse.tile as tile

    f32 = mybir.dt.float32
    f32r = mybir.dt.float32r
    bf16 = mybir.dt.bfloat16
    f16 = mybir.dt.float16
    Exp = mybir.ActivationFunctionType.Exp

    nc = bacc.Bacc("TRN2", target_bir_lowering=False, debug=False)

    q_d = nc.dram_tensor("q2", [K_AUG, HALF], f32r, kind="ExternalInput")
    m_d = nc.dram_tensor("m2", [K_AUG, N], f32r, kind="ExternalInput")
    b_d = nc.dram_tensor("bias", [M_STRIP, N_STRIPS], f32, kind="ExternalInput")
    out_d = nc.dram_tensor("out_c", [HALF, N], bf16, kind="ExternalOutput")

    with tile.TileContext(nc) as tc:
        with (
            tc.tile_pool(name="singles", bufs=1) as singles,
            tc.tile_pool(name="psum", bufs=2, space="PSUM") as psum_pool,
            tc.tile_pool(name="exp", bufs=_EXPB) as exp_pool,
            tc.tile_pool(name="outs", bufs=_OUTB) as out_pool,
            tc.tile_pool(name="stats", bufs=8) as stats_pool,
        ):
            # --- prewarm: ACT exp table load + PE pstate ramp during the
            # input DMAs -----------------------------------------------------
            wtab = singles.tile([1, 2], f32)
            nc.vector.memset(wtab, 0.0)
            nc.scalar.activation(wtab[:, 1:2], wtab[:, 0:1], Exp)
            wsrc = singles.tile([K_AUG, 256], bf16)
            nc.vector.memset(wsrc, 0.0)
            wps = psum_pool.tile([M_STRIP, 2048], f32, tag="ps")
            for _ in range(_SPINS):
                nc.tensor.matmul(
                    wps[:, :256], wsrc[:, :M_STRIP], wsrc, start=True, stop=True
                )

            # --- inputs, staged by first use.  SP ring: bias (tiny, gates the
            # first exp) + strip-0 q + first two m chunks + rest of q; Pool
            # ring (SWDGE, otherwise idle) carries the remaining m chunks so
            # the two sequencers dispatch concurrently and the ACT sequencer
            # stays free for exp dispatches --------------------------------
            q_s = singles.tile([K_AUG, HALF], f32r)
            m_s = singles.tile([K_AUG, N], f32r)
            b_s = singles.tile([M_STRIP, N_STRIPS], f32)
            nc.sync.dma_start(out=b_s, in_=b_d[:, :])
            _q0_ring = {"sp": nc.sync, "pool": nc.gpsimd}[
                _os.environ.get("K_Q0", "sp")
            ]
            _q0_ring.dma_start(out=q_s[:, :M_STRIP], in_=q_d[:, :M_STRIP])
            for c in range(_SYNCM):
                sl = slice(c * N_CHUNK, (c + 1) * N_CHUNK)
                nc.sync.dma_start(out=m_s[:, sl], in_=m_d[:, sl])
            for c in range(_SYNCM, 8):
                sl = slice(c * N_CHUNK, (c + 1) * N_CHUNK)
                nc.gpsimd.dma_start(out=m_s[:, sl], in_=m_d[:, sl])
            _qr_ring = {"sp": nc.sync, "act": nc.scalar, "dve": nc.vector}[
                _os.environ.get("K_QR", "sp")
            ]
            _qr_ring.dma_start(out=q_s[:, M_STRIP:], in_=q_d[:, M_STRIP:])

            for s in range(N_STRIPS):
                m0 = s * M_STRIP
                q_l = q_s[:, m0 : m0 + M_STRIP]

                # strip 0 exps in small leading pieces so the ACT stream
                # starts as soon as the first m chunks land; steady state
                # uses 2016-col pieces (fewer per-call overheads)
                if s == 0 and _S0 == "224":
                    pieces = [range(0, 2), range(2, 4), range(4, 8)]
                elif s == 0 and _S0 == "2222":
                    pieces = [range(0, 2), range(2, 4), range(4, 6), range(6, 8)]
                else:
                    pieces = [range(0, 4), range(4, 8)]

                # middle strips skip the ACT accumulator read (187ns/call)
                # on the first piece: DVE re-reduces those exp values from
                # SBUF instead (DVE has slack; ACT is the bottleneck).  Edge
                # strips keep the fused accum on every piece (short tail).
                dve_red = 0 < s < N_STRIPS - 1
                exp_t = exp_pool.tile([M_STRIP, N], bf16, tag="exp")
                acc = stats_pool.tile([M_STRIP, len(pieces)], f32, tag="acc")

                for pi, piece in enumerate(pieces):
                    k = len(piece)
                    # 1 PSUM bank (512 f32) per 504-col chunk; chunks start on
                    # bank boundaries so PE writes never straddle one
                    ps = psum_pool.tile([M_STRIP, 512 * k], f32, tag="ps")
                    for cc, c in enumerate(piece):
                        nc.tensor.matmul(
                            ps[:, cc * 512 : cc * 512 + N_CHUNK],
                            q_l,
                            m_s[:, c * N_CHUNK : (c + 1) * N_CHUNK],
                            start=True,
                            stop=True,
                        )
                    # exp(logits + bias_m) PSUM->SBUF bf16 with fused
                    # per-partition row sum; 3D views skip the 8 pad cols/bank
                    e0 = piece[0] * N_CHUNK
                    nc.scalar.activation(
                        exp_t[:, e0 : e0 + k * N_CHUNK].rearrange(
                            "p (b c) -> p b c", b=k
                        ),
                        ps.rearrange("p (b c) -> p b c", b=k)[:, :, :N_CHUNK],
                        Exp,
                        bias=b_s[:, s : s + 1],
                        accum_out=None if (dve_red and pi == 0) else acc[:, pi : pi + 1],
                    )

                ssum = stats_pool.tile([M_STRIP, 1], f32, tag="ssum")
                if dve_red:
                    ared = stats_pool.tile([M_STRIP, 1], f32, tag="ared")
                    nc.vector.reduce_sum(
                        ared, exp_t[:, :PIECE], axis=mybir.AxisListType.X
                    )
                    nc.vector.tensor_add(ssum, ared, acc[:, 1:2])
                else:
                    nc.vector.reduce_sum(ssum, acc, axis=mybir.AxisListType.X)
                rcp = stats_pool.tile([M_STRIP, 1], f32, tag="rcp")
                nc.vector.reciprocal(rcp, ssum)

                out_t = out_pool.tile([M_STRIP, N], bf16, tag="out")
                if s == 0:
                    # quarters so the first bytes hit the DMA ring early
                    tsm_bounds = [0, 1008, 2016, 3024, N]
                    store_bounds = tsm_bounds
                elif s == N_STRIPS - 1:
                    # quartered scale + stores shorten the drain tail
                    tsm_bounds = [0, 1008, 2016, 3024, N]
                    store_bounds = tsm_bounds
                else:
                    tsm_bounds = [0, N]
                    store_bounds = [0, 2016, N]
                tsm_spans = dict(zip(tsm_bounds, tsm_bounds[1:]))
                for p0, p1 in zip(store_bounds, store_bounds[1:]):
                    if p0 in tsm_spans:
                        t1 = tsm_spans[p0]
                        nc.vector.tensor_scalar_mul(
                            out_t[:, p0:t1], exp_t[:, p0:t1], rcp
                        )
                    nc.sync.dma_start(
                        out=out_d[m0 : m0 + M_STRIP, p0:p1],
                        in_=out_t[:, p0:p1],
                    )

    nc.compile()
    return nc


def _get_nc():
    if "nc" not in _CACHE:
        _CACHE["nc"] = _build_nc()
    return _CACHE["nc"]


def _round_tf32(x: np.ndarray) -> np.ndarray:
    """Round f32 to 11-bit mantissa (tf32/f32r) with round-to-nearest."""
    xi = np.ascontiguousarray(x, dtype=np.float32).view(np.uint32)
    return ((xi + np.uint32(0x1000)) & np.uint32(0xFFFFE000)).view(np.float32)


def kernel(mk: np.ndarray, qk: np.ndarray) -> np.ndarray:
    from concourse import bass_utils

    mk = np.asarray(mk, dtype=np.float32).reshape(B, CK, N)
    qk = np.asarray(qk, dtype=np.float32).reshape(B, CK, N)
    a = np.einsum("bcn,bcn->bn", mk, mk)        # ||mk_n||^2, [B, N]
    cq = np.einsum("bcm,bcm->bm", qk, qk)       # ||qk_m||^2, [B, M]
    abar = a.mean(axis=1)                       # [B]

    in_maps = []
    for core in range(8):
        b, h = divmod(core, 2)
        m2 = np.empty((K_AUG, N), np.float32)
        m2[:CK] = mk[b]
        m2[CK] = a[b] - abar[b]
        m2 = _round_tf32(m2)

        q2 = np.empty((K_AUG, HALF), np.float32)
        q2[:CK] = 0.25 * qk[b, :, h * HALF : (h + 1) * HALF]
        q2[CK] = -0.125
        q2 = _round_tf32(q2)

        bias = (
            (-0.125 * (cq[b, h * HALF : (h + 1) * HALF] + abar[b]))
            .astype(np.float32)
            .reshape(N_STRIPS, M_STRIP)
            .T.copy()
        )
        in_maps.append({"q2": q2, "m2": m2, "bias": bias})

    res = bass_utils.run_bass_kernel_spmd(
        _get_nc(), in_maps, core_ids=list(range(8))
    )
    _CACHE["last_results"] = res

    out = np.empty((B, N, N), np.float32)
    for core in range(8):
        b, h = divmod(core, 2)
        out[b, :, h * HALF : (h + 1) * HALF] = (
            res.results[core]["out_c"].T.astype(np.float32)
        )
    return out


# revision 42
# speedup vs baseline: 1.0482x; 1.0017x over previous
"""AttentionMemory kernel for Trainium2 (8 NeuronCores, Bass/Tile).

Reference computation (per batch b):
    affinity[n, m] = (2 * mk[:,n]@qk[:,m] - ||mk[:,n]||^2 - ||qk[:,m]||^2) / 8
    out[n, m]      = softmax over n (memory axis)

Logits come from a single float32r (tf32-speed) augmented matmul:
    lhsT (stationary) = [0.25 * qk ; -0.125]      -> [65, Mc]
    rhs  (moving)     = [mk        ; a_n - abar]  -> [65, N]
    psum[m, n]        = 0.25*qk_m.mk_n - 0.125*(a_n - abar)
with a_n = sum_c mk[c,n]^2 precomputed on the host.  The ACT exp pass adds a
per-partition bias -0.125*(||qk_m||^2 + abar), making the exp argument exactly
-||mk_n - qk_m||^2 / 8 <= 0: no overflow, and per-column constants cancel in
the softmax.  float32r rounds inputs to ~tf32 (pre-rounded on the host so
the declared dtype is honest); measured end-to-end absmax error is 3.1e-3
of scale (gate 2e-2).

exp values and the normalized output are staged in bf16 (DVE runs 4x in
16-bit, DMA stores are half-size; bf16's f32-range exponent avoids the
denormal flushing fp16 would hit on tiny softmax outputs); the host
upconverts to f32 during the gather/transpose.  Row sums ride the ACT
accumulator / a DVE re-reduce, reciprocal + scale on DVE.

Sharding: core c handles batch c//2, query-column half c%2 (communication
free: softmax is over the full n axis which each core holds).  Each core
writes out_c[m, n] bf16; the host transposes to the reference [n, m] layout.

Middle strips skip the ACT accumulator read (187ns/call) on their first
2016-col piece; DVE re-reduces those exp values from SBUF instead (DVE has
~25us of slack, ACT is the bottleneck engine).

Per-core budget (TimelineSim cost model): ACT exp stream ~64.7us busy and
gap-free (bottleneck), bf16 stores + f32 loads ~49.7us DMA, PE f32r ~30us,
DVE ~49us.  Total 75.5us = ~5.2us startup + ACT stream + ~6.4us drain tail
(vs 104.5us for the f32/bf16-pair baseline).
"""

import numpy as np

B, CK, H, W = 4, 64, 48, 84
N = H * W            # 4032 memory pixels (softmax axis)
HALF = N // 2        # 2016 query pixels per core
M_STRIP = 126        # output-partition strip size (16 * 126 = 2016)
N_STRIPS = HALF // M_STRIP
K_AUG = CK + 1       # 65: contraction dim incl. the (a_n - abar) row

N_CHUNK = 504        # matmul moving free dim; 4 chunks per 4-bank PSUM piece
PIECE = 4 * N_CHUNK  # 2016 cols per ACT exp call
N_PIECES = N // PIECE  # 2

_CACHE = {}

# scheduler-lottery knobs (neutral semantics, reshuffle the tile scheduler);
# defaults are the shipped configuration
import os as _os
_SPINS = int(_os.environ.get("K_SPINS", "14"))
_EXPB = int(_os.environ.get("K_EXPB", "3"))
_OUTB = int(_os.environ.get("K_OUTB", "3"))
_SYNCM = int(_os.environ.get("K_SYNCM", "2"))
_S0 = _os.environ.get("K_S0", "2222")


def _build_nc():
    import concourse.bacc as bacc
    import concourse.mybir as mybir
    import concourse.tile as tile

    f32 = mybir.dt.float32
    f32r = mybir.dt.float32r
    bf16 = mybir.dt.bfloat16
    f16 = mybir.dt.float16
    Exp = mybir.ActivationFunctionType.Exp

    nc = bacc.Bacc("TRN2", target_bir_lowering=False, debug=False)

    q_d = nc.dram_tensor("q2", [K_AUG, HALF], f32r, kind="ExternalInput")
    m_d = nc.dram_tensor("m2", [K_AUG, N], f32r, kind="ExternalInput")
    b_d = nc.dram_tensor("bias", [M_STRIP, N_STRIPS], f32, kind="ExternalInput")
    out_d = nc.dram_tensor("out_c", [HALF, N], bf16, kind="ExternalOutput")

    with tile.TileContext(nc) as tc:
        with (
            tc.tile_pool(name="singles", bufs=1) as singles,
            tc.tile_pool(name="psum", bufs=2, space="PSUM") as psum_pool,
            tc.tile_pool(name="exp", bufs=_EXPB) as exp_pool,
            tc.tile_pool(name="outs", bufs=_OUTB) as out_pool,
            tc.tile_pool(name="stats", bufs=int(_os.environ.get("K_STB", "8"))) as stats_pool,
        ):
            # --- prewarm: ACT exp table load + PE pstate ramp during the
            # input DMAs -----------------------------------------------------
            wtab = singles.tile([1, 2], f32)
            nc.vector.memset(wtab, 0.0)
            nc.scalar.activation(wtab[:, 1:2], wtab[:, 0:1], Exp)
            wsrc = singles.tile([K_AUG, 256], bf16)
            nc.vector.memset(wsrc, 0.0)
            wps = psum_pool.tile([M_STRIP, 2048], f32, tag="ps")
            for _ in range(_SPINS):
                nc.tensor.matmul(
                    wps[:, :256], wsrc[:, :M_STRIP], wsrc, start=True, stop=True
                )

            # --- inputs, staged by first use.  SP ring: bias (tiny, gates the
            # first exp) + strip-0 q + first two m chunks + rest of q; Pool
            # ring (SWDGE, otherwise idle) carries the remaining m chunks so
            # the two sequencers dispatch concurrently and the ACT sequencer
            # stays free for exp dispatches --------------------------------
            q_s = singles.tile([K_AUG, HALF], f32r)
            m_s = singles.tile([K_AUG, N], f32r)
            b_s = singles.tile([M_STRIP, N_STRIPS], f32)
            nc.sync.dma_start(out=b_s, in_=b_d[:, :])
            _q0_ring = {"sp": nc.sync, "pool": nc.gpsimd}[
                _os.environ.get("K_Q0", "sp")
            ]
            _q0_ring.dma_start(out=q_s[:, :M_STRIP], in_=q_d[:, :M_STRIP])
            for c in range(_SYNCM):
                sl = slice(c * N_CHUNK, (c + 1) * N_CHUNK)
                nc.sync.dma_start(out=m_s[:, sl], in_=m_d[:, sl])
            for c in range(_SYNCM, 8):
                sl = slice(c * N_CHUNK, (c + 1) * N_CHUNK)
                nc.gpsimd.dma_start(out=m_s[:, sl], in_=m_d[:, sl])
            _qr_ring = {"sp": nc.sync, "act": nc.scalar, "dve": nc.vector}[
                _os.environ.get("K_QR", "sp")
            ]
            _qr_ring.dma_start(out=q_s[:, M_STRIP:], in_=q_d[:, M_STRIP:])

            for s in range(N_STRIPS):
                m0 = s * M_STRIP
                q_l = q_s[:, m0 : m0 + M_STRIP]

                # strip 0 exps in small leading pieces so the ACT stream
                # starts as soon as the first m chunks land; steady state
                # uses 2016-col pieces (fewer per-call overheads)
                if s == 0 and _S0 == "224":
                    pieces = [range(0, 2), range(2, 4), range(4, 8)]
                elif s == 0 and _S0 == "2222":
                    pieces = [range(0, 2), range(2, 4), range(4, 6), range(6, 8)]
                else:
                    pieces = [range(0, 4), range(4, 8)]

                # middle strips skip the ACT accumulator read (187ns/call)
                # on the first piece: DVE re-reduces those exp values from
                # SBUF instead (DVE has slack; ACT is the bottleneck).  Edge
                # strips keep the fused accum on every piece (short tail).
                dve_red = 0 < s < N_STRIPS - 1
                exp_t = exp_pool.tile([M_STRIP, N], bf16, tag="exp")
                acc = stats_pool.tile([M_STRIP, len(pieces)], f32, tag="acc")

                for pi, piece in enumerate(pieces):
                    k = len(piece)
                    # 1 PSUM bank (512 f32) per 504-col chunk; chunks start on
                    # bank boundaries so PE writes never straddle one
                    ps = psum_pool.tile([M_STRIP, 512 * k], f32, tag="ps")
                    for cc, c in enumerate(piece):
                        nc.tensor.matmul(
                            ps[:, cc * 512 : cc * 512 + N_CHUNK],
                            q_l,
                            m_s[:, c * N_CHUNK : (c + 1) * N_CHUNK],
                            start=True,
                            stop=True,
                        )
                    # exp(logits + bias_m) PSUM->SBUF bf16 with fused
                    # per-partition row sum; 3D views skip the 8 pad cols/bank
                    e0 = piece[0] * N_CHUNK
                    nc.scalar.activation(
                        exp_t[:, e0 : e0 + k * N_CHUNK].rearrange(
                            "p (b c) -> p b c", b=k
                        ),
                        ps.rearrange("p (b c) -> p b c", b=k)[:, :, :N_CHUNK],
                        Exp,
                        bias=b_s[:, s : s + 1],
                        accum_out=None if (dve_red and pi == 0) else acc[:, pi : pi + 1],
                    )

                ssum = stats_pool.tile([M_STRIP, 1], f32, tag="ssum")
                if dve_red:
                    ared = stats_pool.tile([M_STRIP, 1], f32, tag="ared")
                    nc.vector.reduce_sum(
                        ared, exp_t[:, :PIECE], axis=mybir.AxisListType.X
                    )
                    nc.vector.tensor_add(ssum, ared, acc[:, 1:2])
                else:
                    nc.vector.reduce_sum(ssum, acc, axis=mybir.AxisListType.X)
                rcp = stats_pool.tile([M_STRIP, 1], f32, tag="rcp")
                nc.vector.reciprocal(rcp, ssum)

                out_t = out_pool.tile([M_STRIP, N], bf16, tag="out")
                if s == 0:
                    # quarters so the first bytes hit the DMA ring early
                    tsm_bounds = [0, 1008, 2016, 3024, N]
                    store_bounds = tsm_bounds
                elif s == N_STRIPS - 1:
                    # quartered scale + stores shorten the drain tail
                    tsm_bounds = [0, 1008, 2016, 3024, N]
                    store_bounds = tsm_bounds
                else:
                    tsm_bounds = [0, N]
                    store_bounds = {
                        "h": [0, 2016, N],
                        "q": [0, 1008, 2016, 3024, N],
                        "f": [0, N],
                    }[_os.environ.get("K_MST", "q")]
                tsm_spans = dict(zip(tsm_bounds, tsm_bounds[1:]))
                for p0, p1 in zip(store_bounds, store_bounds[1:]):
                    if p0 in tsm_spans:
                        t1 = tsm_spans[p0]
                        nc.vector.tensor_scalar_mul(
                            out_t[:, p0:t1], exp_t[:, p0:t1], rcp
                        )
                    nc.sync.dma_start(
                        out=out_d[m0 : m0 + M_STRIP, p0:p1],
                        in_=out_t[:, p0:p1],
                    )

    nc.compile()
    return nc


def _get_nc():
    if "nc" not in _CACHE:
        _CACHE["nc"] = _build_nc()
    return _CACHE["nc"]


def _round_tf32(x: np.ndarray) -> np.ndarray:
    """Round f32 to 11-bit mantissa (tf32/f32r) with round-to-nearest."""
    xi = np.ascontiguousarray(x, dtype=np.float32).view(np.uint32)
    return ((xi + np.uint32(0x1000)) & np.uint32(0xFFFFE000)).view(np.float32)


def kernel(mk: np.ndarray, qk: np.ndarray) -> np.ndarray:
    from concourse import bass_utils

    mk = np.asarray(mk, dtype=np.float32).reshape(B, CK, N)
    qk = np.asarray(qk, dtype=np.float32).reshape(B, CK, N)
    a = np.einsum("bcn,bcn->bn", mk, mk)        # ||mk_n||^2, [B, N]
    cq = np.einsum("bcm,bcm->bm", qk, qk)       # ||qk_m||^2, [B, M]
    abar = a.mean(axis=1)                       # [B]

    in_maps = []
    for core in range(8):
        b, h = divmod(core, 2)
        m2 = np.empty((K_AUG, N), np.float32)
        m2[:CK] = mk[b]
        m2[CK] = a[b] - abar[b]
        m2 = _round_tf32(m2)

        q2 = np.empty((K_AUG, HALF), np.float32)
        q2[:CK] = 0.25 * qk[b, :, h * HALF : (h + 1) * HALF]
        q2[CK] = -0.125
        q2 = _round_tf32(q2)

        bias = (
            (-0.125 * (cq[b, h * HALF : (h + 1) * HALF] + abar[b]))
            .astype(np.float32)
            .reshape(N_STRIPS, M_STRIP)
            .T.copy()
        )
        in_maps.append({"q2": q2, "m2": m2, "bias": bias})

    res = bass_utils.run_bass_kernel_spmd(
        _get_nc(), in_maps, core_ids=list(range(8))
    )
    _CACHE["last_results"] = res

    out = np.empty((B, N, N), np.float32)
    for core in range(8):
        b, h = divmod(core, 2)
        out[b, :, h * HALF : (h + 1) * HALF] = (
            res.results[core]["out_c"].T.astype(np.float32)
        )
    return out
